# revision 1
# baseline (speedup 1.0000x reference)
"""Trainium2 Bass kernel for nn_DataEmbedding (embedding_lookup).

Reference computation (for B=32, L=4096, C_IN=7, D=512):
  out = value_emb + pos_emb + temp_emb
  value_emb = TokenEmbedding(x) @ proj_w.T + proj_b   (73+1 tiny conv1d's, k=8)
  pos_emb   = sinusoid_table(L, D)
  temp_emb  = sum of 4 fixed sinusoid-table lookups from x_mark (indices in [0,7))

Key algebraic fusions done on the host (tiny, O(D*PROJ_IN*KS) work):
  * TokenEmbedding+projection collapse into ONE size-8 conv over L:
      value_emb[b,l,d] = sum_{m,c} A[d,m,c] * xpad[b, l+m, c] + proj_b[d]
    with A = einsum(proj_w[:, :511].reshape(D,73,7), kernels[:73]) + c==0 term.
    On device this is a single K=56 matmul per 128-position tile using an
    im2col operand whose rows (7m+c) are just shifted copies of x^T.
  * The 4 temporal lookups become a 28-row one-hot matmul (tables only ever
    indexed at rows 0..6, and all four sinusoid tables agree there).  The
    one-hot rows are stacked under the im2col rows -> ONE K=84 matmul.
  * pos_emb + proj_b are folded into one [L, D] table added during PSUM
    eviction (DVE tensor_tensor add), so bias costs nothing extra.

Sharding: pure data parallel over batch: 32 batches -> 8 cores x 4 batches.
"""

import os
import sys
import ml_dtypes
import numpy as np

for _p in ("/opt/trn_rl_repo", "/opt/pypackages"):
    if os.path.isdir(_p) and _p not in sys.path:
        sys.path.append(_p)

from contextlib import ExitStack

import concourse.bass as bass
import concourse.tile as tile
from concourse import bacc, mybir
from concourse.bass_utils import run_bass_kernel_spmd

# ---------------------------------------------------------------- constants
B, L, C_IN, D = 32, 4096, 7, 512
KS, NK, M = 8, 74, 7          # kernel_size, num_kernels, history
PROJ_IN = 73 * C_IN + 1       # 512
N_CORES = 8
NB = B // N_CORES             # batches per core = 4
R = L + KS                    # padded row length for x^T (4104)
KIM = KS * C_IN               # im2col rows = 56
KOH = 4 * 7                   # one-hot rows = 28
KTOT = KIM + KOH              # fused contraction = 84
P = 128                       # positions per tile
NT = L // P                   # tiles per batch = 32
GT = 8                        # tiles per output-DMA group
NG = NT // GT                 # groups per batch = 4

F32 = mybir.dt.float32


def _sinusoid_table(n, d):
    pos = np.arange(n, dtype=np.float32)[:, None]
    div = np.exp(np.arange(0, d, 2, dtype=np.float32) * (-np.log(10000.0) / d))
    tab = np.zeros((n, d), dtype=np.float32)
    tab[:, 0::2] = np.sin(pos * div)
    tab[:, 1::2] = np.cos(pos * div)
    return tab


_POS_CACHE = None


def _pos_const():
    global _POS_CACHE
    if _POS_CACHE is None:
        _POS_CACHE = _sinusoid_table(L, D)
    return _POS_CACHE


def _host_prep(x, x_mark, kernels, proj_w, proj_b):
    """Build per-core inputs. All heavy math stays on device; this is layout
    glue plus the tiny [512,511]x[73,8] weight fold."""
    x = np.asarray(x, dtype=np.float32)
    x_mark = np.asarray(x_mark)
    kernels = np.asarray(kernels, dtype=np.float32)
    proj_w = np.asarray(proj_w, dtype=np.float32)
    proj_b = np.asarray(proj_b, dtype=np.float32)

    # x^T, left-padded by M zeros along L: [B, 7, R]
    xpadt = np.zeros((B, C_IN, R), dtype=np.float32)
    xpadt[:, :, M : M + L] = x.transpose(0, 2, 1)

    # x_mark as f32 [B, 4, L] (values 0..6 exact in f32)
    xmf = np.ascontiguousarray(x_mark.transpose(0, 2, 1)).astype(np.float32)

    # fused conv weight A[d, m, c]
    p3 = proj_w[:, : 73 * C_IN].reshape(D, 73, C_IN)
    A = np.einsum("dkc,km->dmc", p3, kernels[:73], dtype=np.float32)
    A[:, :, 0] += np.outer(proj_w[:, 511], kernels[73])
    w_pack = A.transpose(1, 2, 0).reshape(KIM, D)  # row 7m+c

    # temporal tables: all four sinusoid tables agree on rows 0..6.
    # one-hot rows go FIRST (DVE writes must start at partition 0).
    tab7 = _sinusoid_table(7, D)  # [7, D]
    wtab = np.concatenate([np.tile(tab7, (4, 1)), w_pack], axis=0)  # [84, D]
    wtab = np.ascontiguousarray(wtab, dtype=np.float32)

    # positional + bias table (bf16: |values| <= ~1, rounding ~2e-3 abs,
    # negligible vs output scale ~22)
    posb = np.ascontiguousarray(
        (_pos_const() + proj_b[None, :]).astype(ml_dtypes.bfloat16)
    )

    kvec = np.tile(np.arange(7, dtype=np.float32), 4).reshape(KOH, 1)
    kvec = np.ascontiguousarray(kvec)

    in_maps = []
    for core in range(N_CORES):
        sl = slice(core * NB, (core + 1) * NB)
        in_maps.append(
            {
                "xpadt": np.ascontiguousarray(xpadt[sl]),
                "xmf": np.ascontiguousarray(xmf[sl]),
                "wtab": wtab,
                "posb": posb,
                "kvec": kvec,
            }
        )
    return in_maps


# ---------------------------------------------------------------- bass build
def build_nc(use_pos=True, psum_bufs=6, stage_bufs=3, dma_engine="sync",
             mm_dtype="f32r"):
    nc = bacc.Bacc("TRN2", target_bir_lowering=False, debug=False)

    MMD = mybir.dt.float32r if mm_dtype == "f32r" else F32
    xpadt_d = nc.dram_tensor("xpadt", (NB, C_IN, R), MMD, kind="ExternalInput")
    xmf_d = nc.dram_tensor("xmf", (NB, 4, L), F32, kind="ExternalInput")
    wtab_d = nc.dram_tensor("wtab", (KTOT, D), MMD, kind="ExternalInput")
    posb_d = nc.dram_tensor("posb", (L, D), mybir.dt.bfloat16,
                            kind="ExternalInput")
    kvec_d = nc.dram_tensor("kvec", (KOH, 1), F32, kind="ExternalInput")
    out_d = nc.dram_tensor("out", (NB, L, D), F32, kind="ExternalOutput")

    with tile.TileContext(nc) as tc, ExitStack() as ctx:
        dma = getattr(nc, dma_engine)
        consts = ctx.enter_context(tc.tile_pool(name="consts", bufs=1))
        lhs_pool = ctx.enter_context(tc.tile_pool(name="lhs", bufs=2))
        idx_pool = ctx.enter_context(tc.tile_pool(name="idx", bufs=2))
        pos_pool = ctx.enter_context(tc.tile_pool(name="pos", bufs=1))
        stage_pool = ctx.enter_context(tc.tile_pool(name="stage", bufs=stage_bufs))
        psum_pool = ctx.enter_context(
            tc.tile_pool(name="psum", bufs=psum_bufs, space="PSUM")
        )

        wtab_s = consts.tile([KTOT, D], MMD, tag="wtab")
        dma.dma_start(wtab_s[:], wtab_d.ap())
        kvec_s = consts.tile([KOH, 1], F32, tag="kvec")
        dma.dma_start(kvec_s[:], kvec_d.ap())

        # positional(+bias) table, resident in SBUF: NG tiles of [128, GT*D]
        pos_tiles = []
        for g in range(NG):
            pt = pos_pool.tile([P, GT * D], mybir.dt.bfloat16, tag=f"pos{g}")
            src = posb_d.ap()[g * GT * P : (g + 1) * GT * P, :]
            src = src.rearrange("(t p) d -> p t d", p=P)
            dma.dma_start(pt[:].rearrange("p (t d) -> p t d", d=D), src)
            pos_tiles.append(pt)

        for b in range(NB):
            # fused stationary operand: rows 0..27 one-hot, 28..83 im2col of x^T.
            # read x^T from HBM once; the 8 shifted copies are SBUF->SBUF.
            xt = idx_pool.tile([C_IN, R], MMD, tag="xt")
            dma.dma_start(xt[:], xpadt_d.ap()[b])
            lhs = lhs_pool.tile([KTOT, L], MMD, tag="lhs")
            for m in range(KS):
                dma.dma_start(
                    lhs[KOH + C_IN * m : KOH + C_IN * (m + 1), :],
                    xt[:, m : m + L],
                )
            idx = idx_pool.tile([KOH, L], F32, tag="idx")
            for j in range(4):
                dma.dma_start(
                    idx[7 * j : 7 * (j + 1), :],
                    xmf_d.ap()[b, j : j + 1, :].partition_broadcast(7),
                )
            nc.vector.tensor_scalar(
                out=lhs[0:KOH, :],
                in0=idx[:],
                scalar1=kvec_s[:],
                scalar2=None,
                op0=mybir.AluOpType.is_equal,
            )

            for g in range(NG):
                stage = stage_pool.tile([P, GT * D], F32, tag="stage")
                for tl in range(GT):
                    t = g * GT + tl
                    ps = psum_pool.tile([P, D], F32, tag="ps")
                    nc.tensor.matmul(
                        ps[:],
                        lhs[:, P * t : P * (t + 1)],
                        wtab_s[:],
                        start=True,
                        stop=True,
                    )
                    if use_pos:
                        nc.vector.tensor_tensor(
                            out=stage[:, D * tl : D * (tl + 1)],
                            in0=ps[:],
                            in1=pos_tiles[g][:, D * tl : D * (tl + 1)],
                            op=mybir.AluOpType.add,
                        )
                    else:
                        nc.vector.tensor_copy(
                            stage[:, D * tl : D * (tl + 1)], ps[:]
                        )
                dst = out_d.ap()[b, g * GT * P : (g + 1) * GT * P, :]
                dst = dst.rearrange("(t p) d -> p t d", p=P)
                dma.dma_start(dst, stage[:].rearrange("p (t d) -> p t d", d=D))

    nc.compile()
    return nc


_NC_CACHE = None


def _get_nc():
    global _NC_CACHE
    if _NC_CACHE is None:
        _NC_CACHE = build_nc()
    return _NC_CACHE


TRACE = False          # set by test.py to capture an NTFF profile
LAST_RESULT = None     # BassKernelResults of the most recent run


def _run_once(in_maps):
    global LAST_RESULT
    nc = _get_nc()
    res = run_bass_kernel_spmd(
        nc, in_maps, core_ids=list(range(N_CORES)), trace=TRACE
    )
    LAST_RESULT = res
    return np.concatenate([r["out"] for r in res.results], axis=0)


def _run_subprocess(inputs):
    """Crash-isolated fallback: run in a fresh interpreter (a device fault can
    wedge the parent process's jax runtime)."""
    import pickle
    import subprocess
    import tempfile

    with tempfile.TemporaryDirectory() as td:
        fin = os.path.join(td, "in.pkl")
        fout = os.path.join(td, "out.npy")
        with open(fin, "wb") as f:
            pickle.dump(inputs, f)
        code = (
            "import pickle, numpy as np, sys;"
            f"sys.path.insert(0, {os.path.dirname(os.path.abspath(__file__))!r});"
            "import kernel as K;"
            f"ins = pickle.load(open({fin!r}, 'rb'));"
            "out = K._run_once(K._host_prep(**ins));"
            f"np.save({fout!r}, out)"
        )
        subprocess.run([sys.executable, "-c", code], check=True, timeout=1800)
        return np.load(fout)


def kernel(x, x_mark, kernels, proj_w, proj_b):
    inputs = dict(x=x, x_mark=x_mark, kernels=kernels, proj_w=proj_w,
                  proj_b=proj_b)
    in_maps = _host_prep(**inputs)
    # the TRN fleet shows rare transient NRT_EXEC_UNIT_UNRECOVERABLE faults;
    # retry in-process first, then in fresh subprocesses.
    for attempt in range(2):
        try:
            return _run_once(in_maps)
        except Exception:
            pass
    for attempt in range(3):
        try:
            return _run_subprocess(inputs)
        except Exception:
            if attempt == 2:
                raise
    raise RuntimeError("unreachable")

